# revision 2
# baseline (speedup 1.0000x reference)
"""CPRINT4Linear on 8 TRN2 NeuronCores — bf16 variant.

Same structure as kernel.py (M-sharded, out^T orientation, flat SW-pipelined
schedule) but weights and activations are bf16 instead of float32r:

- fp32(r) stationary loads get no FWL (fast weight load) and per the
  tensor-engine doc the FP32-HI weight path blocks background-buffer overlap;
  bf16 stationary tiles (128 cols) trigger compiler-automatic FWL and can be
  pulled ahead into the background weight buffer by the PE's 64-deep reorder
  window -> the per-matmul LDWEIGHTS cost hides under the 512-row stream.
- Dequant chain per block: 2 DVE nibble extracts (u8) + 1 fused DVE
  scalar_tensor_tensor (nib - 8) * scale -> bf16 (DEQ_MODE='stt'), or the
  conservative ACT debias + DVE mult chain (DEQ_MODE='act').
- xT shipped/resident in bf16: halves DMA + SBUF footprint.
- out written to DRAM in bf16 (halves the dominant 45MB/pass HBM write
  stream; 8-core runs are HBM-contention-bound). Host casts back to f32.
- Numerics: v2 measured 2.9e-3; bf16 output rounding adds <=1.9e-3.
"""
import numpy as np
import ml_dtypes

import concourse.bacc as bacc
import concourse.mybir as mybir
from concourse.tile import TileContext
from concourse.bass_utils import run_bass_kernel_spmd

B, S, K, N = 4, 2048, 4096, 11008
M = B * S
NCORES = 8
MC = M // NCORES            # 1024 rows per core
G = K // 128                # 32 k-tiles == dequant groups
N_CHUNKS = [512] * 21 + [256]
MT = MC // 128              # 8 m-tiles per core

F32 = mybir.dt.float32
BF16 = mybir.dt.bfloat16
U8 = mybir.dt.uint8

_b = np.arange(G // 2)[:, None, None]
_t = np.arange(2)[None, :, None]
_p = np.arange(128)[None, None, :]
J_ORDER = (256 * _b + 2 * _p + _t).reshape(-1)

TRACE = False
LAST_RESULTS = None
_CACHED_NC = None

DEQ_MODE = "stt"  # 'stt' fused dequant, 'act' conservative chain


def _build(repeats=1, lookahead=6, deq_mode=None):
    deq_mode = deq_mode or DEQ_MODE
    nc = bacc.Bacc("TRN2", target_bir_lowering=False, debug=False, num_devices=NCORES)
    xT = nc.declare_dram_parameter("xT", [K, MC], BF16, isOutput=False)
    wp = nc.declare_dram_parameter("wp", [K // 2, N], U8, isOutput=False)
    sc = nc.declare_dram_parameter("sc", [G, N], BF16, isOutput=False)
    out = nc.declare_dram_parameter("out", [N, MC], BF16, isOutput=True)
    NBK = G // 2  # 16 dequant blocks per chunk (2 k-tiles each)

    with TileContext(nc) as tc:
        with tc.tile_pool(name="xt", bufs=1) as xt_pool, \
             tc.tile_pool(name="wpp", bufs=6) as wp_pool, \
             tc.tile_pool(name="nib", bufs=6) as nib_pool, \
             tc.tile_pool(name="deb", bufs=3) as deb_pool, \
             tc.tile_pool(name="wf", bufs=4) as wf_pool, \
             tc.tile_pool(name="scb", bufs=4) as sc_pool, \
             tc.tile_pool(name="ob", bufs=8) as out_pool, \
             tc.tile_pool(name="ps", bufs=8, space="PSUM") as psum_pool:

            xts = [None] * G  # resident activations, loaded just-in-time

            seq = []
            for rep in range(repeats):
                n0 = 0
                for ci, nsz in enumerate(N_CHUNKS):
                    for b in range(NBK):
                        seq.append((rep, ci, n0, nsz, b))
                    n0 += nsz

            wfts = {}    # flat index -> wft block tile [128, 2, nsz]
            ptiles = {}  # (rep, ci) -> list of psum tiles

            def produce(i):
                rep, ci, n0, nsz, b = seq[i]
                if rep == 0 and ci == 0:
                    for g in (2 * b, 2 * b + 1):
                        if xts[g] is None:
                            t = xt_pool.tile([128, MC], BF16, tag=f"xt{g}",
                                             name=f"xt{g}")
                            nc.scalar.dma_start(
                                out=t[:], in_=xT[128 * g:128 * (g + 1), :])
                            xts[g] = t
                # scale rows 2b (partitions 0..63) and 2b+1 (64..127)
                sct = sc_pool.tile([128, nsz], BF16, name="sct")
                nc.scalar.dma_start(
                    out=sct[0:64, :],
                    in_=sc[2 * b:2 * b + 1, n0:n0 + nsz].to_broadcast([64, nsz]))
                nc.scalar.dma_start(
                    out=sct[64:128, :],
                    in_=sc[2 * b + 1:2 * b + 2, n0:n0 + nsz].to_broadcast([64, nsz]))
                # packed rows [128b, 128b+128), plain load
                wpt = wp_pool.tile([128, nsz], U8, name="wpt")
                nc.sync.dma_start(out=wpt[:],
                                  in_=wp[128 * b:128 * (b + 1), n0:n0 + nsz])
                # nibble extract into pair tile: [:,0,:] low, [:,1,:] high
                nib = nib_pool.tile([128, 2, nsz], U8, name="nib")
                nc.vector.tensor_scalar(out=nib[:, 0, :], in0=wpt[:],
                                        scalar1=15, scalar2=None,
                                        op0=mybir.AluOpType.bitwise_and)
                nc.vector.tensor_scalar(out=nib[:, 1, :], in0=wpt[:],
                                        scalar1=4, scalar2=None,
                                        op0=mybir.AluOpType.logical_shift_right)
                wft = wf_pool.tile([128, 2, nsz], BF16, name="wft")
                if deq_mode == "stt":
                    # fused (nib - 8) * scale on DVE
                    nc.vector.scalar_tensor_tensor(
                        out=wft[:], in0=nib[:], scalar=-8.0,
                        in1=sct[:].unsqueeze(1).to_broadcast([128, 2, nsz]),
                        op0=mybir.AluOpType.add,
                        op1=mybir.AluOpType.mult)
                else:
                    deb = deb_pool.tile([128, 2, nsz], BF16, name="deb")
                    nc.scalar.activation(deb[:], nib[:],
                                         mybir.ActivationFunctionType.Copy,
                                         bias=-8.0, scale=1.0)
                    nc.vector.tensor_tensor(
                        out=wft[:], in0=deb[:],
                        in1=sct[:].unsqueeze(1).to_broadcast([128, 2, nsz]),
                        op=mybir.AluOpType.mult)
                wfts[i] = wft

            MH = MC // 512  # moving m-chunks per k-tile (2)

            def consume(i):
                rep, ci, n0, nsz, b = seq[i]
                nt_cnt = nsz // 128
                # lhsT = w (stationary, reused across 2 m-chunks),
                # rhs = xT (moving 512 wide), psum holds out^T [128 n, 512 m]
                if b == 0:
                    ptiles[(rep, ci)] = [
                        psum_pool.tile([128, 512], F32, name="ps", tag="ps")
                        for _ in range(nt_cnt * MH)]
                pts = ptiles[(rep, ci)]
                wft = wfts.pop(i)
                for t_half in range(2):
                    g = 2 * b + t_half
                    for nt in range(nt_cnt):
                        for h in range(MH):
                            nc.tensor.matmul(
                                pts[nt * MH + h][:],
                                wft[:, t_half, 128 * nt:128 * (nt + 1)],
                                xts[g][:, 512 * h:512 * (h + 1)],
                                start=(g == 0), stop=(g == G - 1))
                if b == NBK - 1:
                    for nt in range(nt_cnt):
                        for h in range(MH):
                            ot = out_pool.tile([128, 512], BF16, name="ot")
                            if (nt + h) % 2 == 0:
                                nc.vector.tensor_copy(ot[:], pts[nt * MH + h][:])
                            else:
                                nc.scalar.activation(
                                    ot[:], pts[nt * MH + h][:],
                                    mybir.ActivationFunctionType.Copy)
                            nc.sync.dma_start(
                                out=out[n0 + 128 * nt:n0 + 128 * (nt + 1),
                                        512 * h:512 * (h + 1)],
                                in_=ot[:])
                    del ptiles[(rep, ci)]

            for i in range(min(lookahead, len(seq))):
                produce(i)
            for i in range(len(seq)):
                if i + lookahead < len(seq):
                    produce(i + lookahead)
                consume(i)
    nc.compile()
    return nc


def _prep_inputs(x, col_indices, w_packed, scales):
    x2 = np.ascontiguousarray(np.asarray(x, dtype=np.float32).reshape(M, K))
    perm = np.asarray(col_indices).astype(np.int64)[J_ORDER]
    wp_u8 = np.asarray(w_packed).astype(np.uint8)
    sc_b = np.asarray(scales, dtype=np.float32).astype(ml_dtypes.bfloat16)
    in_maps = []
    for c in range(NCORES):
        xTc = np.ascontiguousarray(
            x2[c * MC:(c + 1) * MC, perm].T.astype(ml_dtypes.bfloat16))
        in_maps.append({"xT": xTc, "wp": wp_u8, "sc": sc_b})
    return in_maps


def kernel(x, col_indices, w_packed, scales, bias):
    global LAST_RESULTS, _CACHED_NC
    if _CACHED_NC is None:
        _CACHED_NC = _build()
    nc = _CACHED_NC

    in_maps = _prep_inputs(x, col_indices, w_packed, scales)
    res = run_bass_kernel_spmd(nc, in_maps, list(range(NCORES)), trace=TRACE)
    LAST_RESULTS = res

    out = np.concatenate(
        [np.ascontiguousarray(res.results[c]["out"].T).astype(np.float32)
         for c in range(NCORES)],
        axis=0)
    out = out + np.asarray(bias, dtype=np.float32)[None, :]
    return np.ascontiguousarray(out.reshape(B, S, N).astype(np.float32))


# revision 3
# speedup vs baseline: 1.0889x; 1.0889x over previous
"""CPRINT4Linear on 8 TRN2 NeuronCores.

out[M,N] = gather_cols(x)[M,K] @ dequant_int4(w_packed)[K,N] + bias

Strategy (M-sharded data parallel, out^T orientation, bf16 datapath):
- Shard M (=B*S=8192) across 8 cores: core c computes out rows
  [1024c, 1024(c+1)). No collectives; host concatenates the shards.
- Host folds the col_indices gather + transpose + nibble-deinterleave into
  one permuted transpose of x (J_ORDER), shipped as bf16 xT [K, MC] and
  kept resident in SBUF (64KB/partition).
- Per 512-wide n-chunk: 16 dequant blocks (2 k-tiles each): plain wp u8
  load -> 2 DVE nibble extracts -> 1 fused DVE scalar_tensor_tensor
  (nib - 8) * scale -> bf16 weight tile; then matmuls with the bf16 weight
  as stationary and 512-wide bf16 xT slices as moving, accumulating out^T
  [128n, 512m] in fp32 across all 8 PSUM banks over the 32 k-tiles.
  PSUM -> SBUF copyback casts to bf16 (alternating DVE/ACT), DMA'd to a
  bf16 out [N, MC]; host transposes + casts to f32 and adds bias exactly.
- Software-pipelined flat schedule: dequant blocks produced `lookahead`
  blocks ahead of their matmul consumption.

Why bf16 everywhere: PE microbenchmarks show the matmul stream runs at the
~213ns/512-row ideal for both f32r and bf16 (weight loads fully hidden), so
per-core compute is already at the 1.17ms roofline (1-core measures 1.23ms).
The 8-core slowdown is inter-core HBM/fabric contention; bf16 x, weights and
OUTPUT cut per-core per-pass HBM traffic from ~68MB (fp32r baseline) to
~45MB, which recovered most of the contention loss (1.57 -> 1.31 ms/pass
measured back-to-back; rel err 4.4e-3 vs the 2e-2 gate).
"""
import numpy as np
import ml_dtypes

import concourse.bacc as bacc
import concourse.mybir as mybir
from concourse.tile import TileContext
from concourse.bass_utils import run_bass_kernel_spmd

B, S, K, N = 4, 2048, 4096, 11008
M = B * S
NCORES = 8
MC = M // NCORES            # 1024 rows per core
G = K // 128                # 32 k-tiles == dequant groups
N_CHUNKS = [512] * 21 + [256]
MT = MC // 128              # 8 m-tiles per core

F32 = mybir.dt.float32
BF16 = mybir.dt.bfloat16
U8 = mybir.dt.uint8

_b = np.arange(G // 2)[:, None, None]
_t = np.arange(2)[None, :, None]
_p = np.arange(128)[None, None, :]
J_ORDER = (256 * _b + 2 * _p + _t).reshape(-1)

TRACE = False
LAST_RESULTS = None
_CACHED_NC = None

DEQ_MODE = "stt"  # 'stt' fused dequant, 'act' conservative chain


def _build(repeats=1, lookahead=6, deq_mode=None):
    deq_mode = deq_mode or DEQ_MODE
    nc = bacc.Bacc("TRN2", target_bir_lowering=False, debug=False, num_devices=NCORES)
    xT = nc.declare_dram_parameter("xT", [K, MC], BF16, isOutput=False)
    wp = nc.declare_dram_parameter("wp", [K // 2, N], U8, isOutput=False)
    sc = nc.declare_dram_parameter("sc", [G, N], BF16, isOutput=False)
    out = nc.declare_dram_parameter("out", [N, MC], BF16, isOutput=True)
    NBK = G // 2  # 16 dequant blocks per chunk (2 k-tiles each)

    with TileContext(nc) as tc:
        with tc.tile_pool(name="xt", bufs=1) as xt_pool, \
             tc.tile_pool(name="wpp", bufs=6) as wp_pool, \
             tc.tile_pool(name="nib", bufs=6) as nib_pool, \
             tc.tile_pool(name="deb", bufs=3) as deb_pool, \
             tc.tile_pool(name="wf", bufs=4) as wf_pool, \
             tc.tile_pool(name="scb", bufs=4) as sc_pool, \
             tc.tile_pool(name="ob", bufs=8) as out_pool, \
             tc.tile_pool(name="ps", bufs=8, space="PSUM") as psum_pool:

            xts = [None] * G  # resident activations, loaded just-in-time

            seq = []
            for rep in range(repeats):
                n0 = 0
                for ci, nsz in enumerate(N_CHUNKS):
                    for b in range(NBK):
                        seq.append((rep, ci, n0, nsz, b))
                    n0 += nsz

            wfts = {}    # flat index -> wft block tile [128, 2, nsz]
            ptiles = {}  # (rep, ci) -> list of psum tiles

            def produce(i):
                rep, ci, n0, nsz, b = seq[i]
                if rep == 0 and ci == 0:
                    for g in (2 * b, 2 * b + 1):
                        if xts[g] is None:
                            t = xt_pool.tile([128, MC], BF16, tag=f"xt{g}",
                                             name=f"xt{g}")
                            nc.scalar.dma_start(
                                out=t[:], in_=xT[128 * g:128 * (g + 1), :])
                            xts[g] = t
                # scale rows 2b (partitions 0..63) and 2b+1 (64..127)
                sct = sc_pool.tile([128, nsz], BF16, name="sct")
                nc.scalar.dma_start(
                    out=sct[0:64, :],
                    in_=sc[2 * b:2 * b + 1, n0:n0 + nsz].to_broadcast([64, nsz]))
                nc.scalar.dma_start(
                    out=sct[64:128, :],
                    in_=sc[2 * b + 1:2 * b + 2, n0:n0 + nsz].to_broadcast([64, nsz]))
                # packed rows [128b, 128b+128), plain load
                wpt = wp_pool.tile([128, nsz], U8, name="wpt")
                nc.sync.dma_start(out=wpt[:],
                                  in_=wp[128 * b:128 * (b + 1), n0:n0 + nsz])
                # nibble extract into pair tile: [:,0,:] low, [:,1,:] high
                nib = nib_pool.tile([128, 2, nsz], U8, name="nib")
                nc.vector.tensor_scalar(out=nib[:, 0, :], in0=wpt[:],
                                        scalar1=15, scalar2=None,
                                        op0=mybir.AluOpType.bitwise_and)
                nc.vector.tensor_scalar(out=nib[:, 1, :], in0=wpt[:],
                                        scalar1=4, scalar2=None,
                                        op0=mybir.AluOpType.logical_shift_right)
                wft = wf_pool.tile([128, 2, nsz], BF16, name="wft")
                if deq_mode == "stt":
                    # fused (nib - 8) * scale on DVE
                    nc.vector.scalar_tensor_tensor(
                        out=wft[:], in0=nib[:], scalar=-8.0,
                        in1=sct[:].unsqueeze(1).to_broadcast([128, 2, nsz]),
                        op0=mybir.AluOpType.add,
                        op1=mybir.AluOpType.mult)
                else:
                    deb = deb_pool.tile([128, 2, nsz], BF16, name="deb")
                    nc.scalar.activation(deb[:], nib[:],
                                         mybir.ActivationFunctionType.Copy,
                                         bias=-8.0, scale=1.0)
                    nc.vector.tensor_tensor(
                        out=wft[:], in0=deb[:],
                        in1=sct[:].unsqueeze(1).to_broadcast([128, 2, nsz]),
                        op=mybir.AluOpType.mult)
                wfts[i] = wft

            MH = MC // 512  # moving m-chunks per k-tile (2)

            def consume(i):
                rep, ci, n0, nsz, b = seq[i]
                nt_cnt = nsz // 128
                # lhsT = w (stationary, reused across 2 m-chunks),
                # rhs = xT (moving 512 wide), psum holds out^T [128 n, 512 m]
                if b == 0:
                    ptiles[(rep, ci)] = [
                        psum_pool.tile([128, 512], F32, name="ps", tag="ps")
                        for _ in range(nt_cnt * MH)]
                pts = ptiles[(rep, ci)]
                wft = wfts.pop(i)
                for t_half in range(2):
                    g = 2 * b + t_half
                    for nt in range(nt_cnt):
                        for h in range(MH):
                            nc.tensor.matmul(
                                pts[nt * MH + h][:],
                                wft[:, t_half, 128 * nt:128 * (nt + 1)],
                                xts[g][:, 512 * h:512 * (h + 1)],
                                start=(g == 0), stop=(g == G - 1))
                if b == NBK - 1:
                    for nt in range(nt_cnt):
                        for h in range(MH):
                            ot = out_pool.tile([128, 512], BF16, name="ot")
                            if (nt + h) % 2 == 0:
                                nc.vector.tensor_copy(ot[:], pts[nt * MH + h][:])
                            else:
                                nc.scalar.activation(
                                    ot[:], pts[nt * MH + h][:],
                                    mybir.ActivationFunctionType.Copy)
                            nc.sync.dma_start(
                                out=out[n0 + 128 * nt:n0 + 128 * (nt + 1),
                                        512 * h:512 * (h + 1)],
                                in_=ot[:])
                    del ptiles[(rep, ci)]

            for i in range(min(lookahead, len(seq))):
                produce(i)
            for i in range(len(seq)):
                if i + lookahead < len(seq):
                    produce(i + lookahead)
                consume(i)
    nc.compile()
    return nc


def _prep_inputs(x, col_indices, w_packed, scales):
    x2 = np.ascontiguousarray(np.asarray(x, dtype=np.float32).reshape(M, K))
    perm = np.asarray(col_indices).astype(np.int64)[J_ORDER]
    wp_u8 = np.asarray(w_packed).astype(np.uint8)
    sc_b = np.asarray(scales, dtype=np.float32).astype(ml_dtypes.bfloat16)
    in_maps = []
    for c in range(NCORES):
        xTc = np.ascontiguousarray(
            x2[c * MC:(c + 1) * MC, perm].T.astype(ml_dtypes.bfloat16))
        in_maps.append({"xT": xTc, "wp": wp_u8, "sc": sc_b})
    return in_maps


def kernel(x, col_indices, w_packed, scales, bias):
    global LAST_RESULTS, _CACHED_NC
    if _CACHED_NC is None:
        _CACHED_NC = _build()
    nc = _CACHED_NC

    in_maps = _prep_inputs(x, col_indices, w_packed, scales)
    res = run_bass_kernel_spmd(nc, in_maps, list(range(NCORES)), trace=TRACE)
    LAST_RESULTS = res

    out = np.concatenate(
        [np.ascontiguousarray(res.results[c]["out"].T).astype(np.float32)
         for c in range(NCORES)],
        axis=0)
    out = out + np.asarray(bias, dtype=np.float32)[None, :]
    return np.ascontiguousarray(out.reshape(B, S, N).astype(np.float32))
